# revision 2
# baseline (speedup 1.0000x reference)
"""Trainium2 Bass kernel for ColumnConsistencyLoss (segment_reduce).

Problem: B=16, T=8192, C=128.
  probs = softmax(logits, -1)           # (N, C), N = B*T = 131072
  per column-id c (segment): n_c = #valid tokens, S_c = sum w*p, Q_c = sum w*p^2
  col_var_c = (sum_j Q_cj - sum_j S_cj^2 / n_safe_c) / (n_safe_c * C)
  loss = mean over columns with n_c > 1 of col_var_c

Sharding (v4, "compacted"): only VALID tokens (w=1, ~50% of N) contribute
to the loss, so the host gathers the valid tokens and distributes them
evenly over the 8 cores (any token->core mapping is legal: the segment
sums are permutation invariant).  Each core gets a fixed capacity of
J=65 tiles x 128 tokens = 8320 tokens (valid count is ~8192 +- 23 per
core); unused slots are padded with zero logits and a zero one-hot
column so they contribute nothing.  In the (astronomically unlikely)
case that the valid count exceeds the total capacity, the overflow
tokens are folded in exactly on the host.

Device kernel per core (tokens on partitions, C on the free dim,
"c-major" within each chunk so the softmax normalizer broadcast hits
the DVE 2x mode):
  - DMA L chunk [P, C, cj] bf16 (c-major, host-packed) + M chunk
    [P, cj, C] fp8 one-hot (row-major for LDWEIGHTS)
  - ACT:  E = exp(L)                  (one big-free instruction)
  - ACT:  E2 = Square(E) on some chunks (same act table as exp)
  - DVE:  d = rowsum(E) via bf16 halving tree h1,h2,h3 (2x mode) +
          strided reduce over the remaining 16
  - DVE:  rv = 1/d (reciprocal_approx_fast), rb = bf16(rv)
  - DVE:  RHS[:, 0:C, :]  = E  * bcast(rb)      (2x mode)
          RHS[:, C:2C, :] = p*p  or  E2*bcast(rb^2)
  - PE:   per tile jj: psum[c, 0:2C] += M[:,jj,:]^T @ RHS[:, :, jj]
          accumulating S rows (F 0:C) and Q rows (F C:2C) over all tiles
Host finalizes: exact n via bincount, ssd = rowsum(Q) - rowsum(S^2)/n,
masked mean over columns with n>1.
"""

import numpy as np
import ml_dtypes

NCORES = 8
P = 128           # partitions (tokens per tile)
C = 128           # columns / segments
B, T = 16, 8192
N_TOK = B * T
CHUNKS = (4, 8, 12, 14, 14, 13)       # token tiles per chunk
ACT_SQ = (False, False, False, True, True, True)  # square on ACT for chunk?
J = sum(CHUNKS)                        # 65 tiles
TOK_CAP = J * P                        # 8320 tokens per core
F2 = 2 * C

TRACE = False          # set True (e.g. from test.py) to capture NTFF profile
TRACE_TMPDIR = None    # where trace/NEFF artifacts land when TRACE is set
LAST_RESULT = None     # BassKernelResults of the last run (for profiling)

_NC_CACHE = {}


def build_nc(chunks=CHUNKS, act_sq=ACT_SQ):
    """Build + compile the Bass program (SPMD; same NEFF on all cores)."""
    from concourse import bacc, mybir
    import concourse.tile as tile

    f32 = mybir.dt.float32
    bf16 = mybir.dt.bfloat16
    fp8 = mybir.dt.float8e4
    Exp = mybir.ActivationFunctionType.Exp
    Square = mybir.ActivationFunctionType.Square
    Copy = mybir.ActivationFunctionType.Copy
    Alu = mybir.AluOpType

    j_full = sum(chunks)
    nchunk = len(chunks)
    offs = [sum(chunks[:k]) for k in range(nchunk)]

    nc = bacc.Bacc("TRN2", target_bir_lowering=False, debug=False,
                   enable_asserts=False)

    lg_d = nc.dram_tensor("logits", [j_full * P * C], bf16,
                          kind="ExternalInput")
    m_d = nc.dram_tensor("m8", [j_full * P * C], fp8, kind="ExternalInput")
    sq_d = nc.dram_tensor("sq_out", [C, F2], f32, kind="ExternalOutput")

    with tile.TileContext(nc) as tc:
        with (
            tc.tile_pool(name="const", bufs=1) as constp,
            tc.tile_pool(name="ld", bufs=4) as ldp,
            tc.tile_pool(name="big", bufs=3) as bigp,
            tc.tile_pool(name="tree", bufs=3) as treep,
            tc.tile_pool(name="small", bufs=4) as smallp,
            tc.tile_pool(name="psum", bufs=1, space="PSUM") as psump,
        ):
            psum_sq = psump.tile([C, F2], f32)

            Ls = [None] * nchunk
            Ms = [None] * nchunk
            Es = [None] * nchunk
            E2s = [None] * nchunk

            def emit_load(k):
                cj = chunks[k]
                base = offs[k] * P * C
                L = ldp.tile([P, C, cj], bf16, tag="L")
                lg_ap = lg_d[base:base + P * C * cj].rearrange(
                    "(p c j) -> p c j", c=C, j=cj)
                nc.sync.dma_start(L[:], lg_ap)
                M8 = ldp.tile([P, cj, C], fp8, tag="M8")
                m_ap = m_d[base:base + P * C * cj].rearrange(
                    "(p j c) -> p j c", j=cj, c=C)
                # scalar-issued DMA rides the second HWDGE ring
                nc.scalar.dma_start(M8[:], m_ap)
                Ls[k], Ms[k] = L, M8

            def emit_exp(k):
                cj = chunks[k]
                E = bigp.tile([P, C, cj], bf16, tag="E")
                nc.scalar.activation(E[:], Ls[k][:], Exp)
                Es[k] = E
                if act_sq[k]:
                    E2 = bigp.tile([P, C, cj], bf16, tag="E2")
                    nc.scalar.activation(E2[:], E[:], Square)
                    E2s[k] = E2

            emit_load(0)
            emit_load(1)
            emit_exp(0)
            for k, cj in enumerate(chunks):
                if k + 2 < nchunk:
                    emit_load(k + 2)
                if k + 1 < nchunk:
                    emit_exp(k + 1)
                E, M8 = Es[k], Ms[k]

                # d = rowsum(E): bf16 halving tree (2x mode) + strided tail
                h1 = treep.tile([P, 64, cj], bf16, tag="h1")
                nc.vector.tensor_tensor(h1[:], E[:, 0:64, :], E[:, 64:128, :],
                                        op=Alu.add)
                h2 = treep.tile([P, 32, cj], bf16, tag="h2")
                nc.vector.tensor_tensor(h2[:], h1[:, 0:32, :], h1[:, 32:64, :],
                                        op=Alu.add)
                h3 = treep.tile([P, 16, cj], bf16, tag="h3")
                nc.vector.tensor_tensor(h3[:], h2[:, 0:16, :], h2[:, 16:32, :],
                                        op=Alu.add)
                d = smallp.tile([P, cj], f32, tag="d")
                nc.vector.tensor_reduce(d[:], h3[:].rearrange("p c j -> p j c"),
                                        axis=mybir.AxisListType.X, op=Alu.add)
                rv = smallp.tile([P, cj], f32, tag="rv")
                nc.vector.reciprocal_approx_fast(rv[:], d[:])
                rb = smallp.tile([P, cj], bf16, tag="rb")
                nc.scalar.activation(rb[:], rv[:], Copy)

                rhs = bigp.tile([P, F2, cj], bf16, tag="rhs")
                nc.vector.tensor_tensor(
                    rhs[:, 0:C, :], E[:],
                    rb[:, None, :].to_broadcast([P, C, cj]),
                    op=Alu.mult)
                if act_sq[k]:
                    r2 = smallp.tile([P, cj], bf16, tag="r2")
                    nc.vector.tensor_tensor(r2[:], rb[:], rb[:], op=Alu.mult)
                    nc.vector.tensor_tensor(
                        rhs[:, C:F2, :], E2s[k][:],
                        r2[:, None, :].to_broadcast([P, C, cj]),
                        op=Alu.mult)
                else:
                    nc.vector.tensor_tensor(
                        rhs[:, C:F2, :], rhs[:, 0:C, :], rhs[:, 0:C, :],
                        op=Alu.mult)

                for jj in range(cj):
                    j = offs[k] + jj
                    nc.tensor.matmul(
                        psum_sq[:], M8[:, jj, :], rhs[:, :, jj],
                        start=(j == 0), stop=(j == j_full - 1))

            out_t = constp.tile([C, F2], f32)
            nc.vector.tensor_copy(out_t[:], psum_sq[:])
            nc.sync.dma_start(sq_d[:], out_t[:])

    nc.compile()
    return nc


def _get_nc():
    key = (CHUNKS, ACT_SQ)
    if key not in _NC_CACHE:
        _NC_CACHE[key] = build_nc(CHUNKS, ACT_SQ)
    return _NC_CACHE[key]


def _pack_core(Lv, Sv, chunks):
    """Pack one core's [J, P, C] logits + [J, P] segments into the DMA
    layouts: logits c-major (p, c, j) per chunk, one-hot row-major
    (p, j, c) per chunk."""
    bf16 = ml_dtypes.bfloat16
    fp8 = ml_dtypes.float8_e4m3
    offs = 0
    lparts = []
    mparts = []
    for cj in chunks:
        Lc = Lv[offs:offs + cj]                      # [cj, P, C]
        lparts.append(np.ascontiguousarray(
            Lc.transpose(1, 2, 0)).ravel())          # (p, c, j)
        M = np.zeros((cj, P, C), dtype=fp8)
        Sc = Sv[offs:offs + cj]                      # [cj, P]
        valid = Sc >= 0
        jj, pp = np.nonzero(valid)
        M[jj, pp, Sc[jj, pp]] = fp8(1.0)
        mparts.append(np.ascontiguousarray(
            M.transpose(1, 0, 2)).ravel())           # (p, j, c)
        offs += cj
    return (np.concatenate(lparts).astype(bf16, copy=False),
            np.concatenate(mparts))


def kernel(column_logits, column_assignments, valid_mask):
    global LAST_RESULT
    from concourse.bass_utils import run_bass_kernel_spmd

    bf16 = ml_dtypes.bfloat16

    logits = np.asarray(column_logits, dtype=np.float32).reshape(N_TOK, C)
    seg = np.asarray(column_assignments).reshape(N_TOK).astype(np.int64)
    w = np.asarray(valid_mask).reshape(N_TOK).astype(bool)

    vidx = np.nonzero(w)[0]
    cap = NCORES * TOK_CAP
    dev_idx = vidx[:cap]
    ov_idx = vidx[cap:]          # overflow (essentially never non-empty)

    nv = dev_idx.size
    # Compacted per-core arrays [J, P, C] / [J, P]; seg = -1 marks padding.
    Lv = np.zeros((cap, C), dtype=bf16)
    Lv[:nv] = logits[dev_idx].astype(bf16)
    Sv = np.full(cap, -1, dtype=np.int64)
    Sv[:nv] = seg[dev_idx]

    in_maps = []
    for i in range(NCORES):
        sl = slice(i * TOK_CAP, (i + 1) * TOK_CAP)
        lpk, mpk = _pack_core(Lv[sl].reshape(J, P, C),
                              Sv[sl].reshape(J, P), CHUNKS)
        in_maps.append({"logits": lpk, "m8": mpk})

    nc = _get_nc()
    res = run_bass_kernel_spmd(nc, in_maps, list(range(NCORES)), trace=TRACE,
                               tmpdir=TRACE_TMPDIR)
    LAST_RESULT = res

    SQ = np.zeros((C, F2), np.float64)
    for rm in res.results:
        SQ += np.asarray(rm["sq_out"], dtype=np.float64)
    S = SQ[:, 0:C].copy()
    Q = SQ[:, C:F2].copy()

    if ov_idx.size:              # exact host fold-in of overflow tokens
        Lo = logits[ov_idx].astype(np.float64)
        Eo = np.exp(Lo)
        po = Eo / Eo.sum(axis=1, keepdims=True)
        so = seg[ov_idx]
        np.add.at(S, so, po)
        np.add.at(Q, so, po * po)

    n = np.bincount(seg[w], minlength=C).astype(np.float64)
    n_safe = np.maximum(n, 1.0)
    ssd_sum = Q.sum(axis=1) - (S * S).sum(axis=1) / n_safe
    col_var = ssd_sum / (n_safe * C)
    has_multi = n > 1.0
    count = has_multi.sum()
    total = np.where(has_multi, col_var, 0.0).sum()
    loss = total / max(count, 1.0) if count > 0 else 0.0
    return np.asarray(loss, dtype=np.float32)


# revision 3
# speedup vs baseline: 1.2634x; 1.2634x over previous
"""Trainium2 Bass kernel for ColumnConsistencyLoss (segment_reduce).

Problem: B=16, T=8192, C=128.
  probs = softmax(logits, -1)           # (N, C), N = B*T = 131072
  per column-id c (segment): n_c = #valid tokens, S_c = sum w*p, Q_c = sum w*p^2
  col_var_c = (sum_j Q_cj - sum_j S_cj^2 / n_safe_c) / (n_safe_c * C)
  loss = mean over columns with n_c > 1 of col_var_c

Sharding ("compacted"): only VALID tokens (w=1, ~50% of N) contribute to
the loss, so the host gathers the valid tokens and distributes them
evenly over the 8 cores (any token->core mapping is legal: the segment
sums are permutation invariant).  Each core gets a fixed capacity of
J=65 tiles x 128 tokens = 8320 tokens (valid count is ~8192 +- 23 per
core); unused slots are padded with zero logits and a zero one-hot
column so they contribute nothing.  In the (astronomically unlikely)
case that the valid count exceeds the total capacity, the overflow
tokens are folded in exactly on the host.

Device kernel per core (tokens on partitions, row-major [P, j, C] tiles
so the matmul moving operand is contiguous):
  - DMA L chunk [P, cj, C] bf16 + M chunk [P, cj, C] fp8 one-hot
  - ACT:  E = exp(L)                  (one big-free instruction)
  - DVE:  d = rowsum(E) via bf16 halving tree h1,h2,h3 (2x mode) +
          contiguous reduce over the remaining 16
  - DVE:  rv = 1/d (reciprocal_approx_fast); ACT: rb = bf16(rv)
  - DVE:  rhs[:, :, 0:C] = E * bcast(rb)        (normalized probs p)
  - ACT or DVE (per-chunk knob): rhs[:, :, C:2C] = p^2
  - PE:   per tile jj: psum[c, 0:2C] += M[:,jj,:]^T @ rhs[:, jj, :]
          accumulating S rows (F 0:C) and Q rows (F C:2C) over all tiles
Host finalizes: exact n via bincount, ssd = rowsum(Q) - rowsum(S^2)/n,
masked mean over columns with n>1.
"""

import numpy as np
import ml_dtypes

NCORES = 8
P = 128           # partitions (tokens per tile)
C = 128           # columns / segments
B, T = 16, 8192
N_TOK = B * T
CHUNKS = (4, 8, 12, 14, 14, 13)       # token tiles per chunk
ACT_SQ = (True, True, True, True, True, True)  # square on ACT for chunk?
J = sum(CHUNKS)                        # 65 tiles
TOK_CAP = J * P                        # 8320 tokens per core
F2 = 2 * C

TRACE = False          # set True (e.g. from test.py) to capture NTFF profile
TRACE_TMPDIR = None    # where trace/NEFF artifacts land when TRACE is set
LAST_RESULT = None     # BassKernelResults of the last run (for profiling)

_NC_CACHE = {}


def build_nc(chunks=CHUNKS, act_sq=ACT_SQ):
    """Build + compile the Bass program (SPMD; same NEFF on all cores)."""
    from concourse import bacc, mybir
    import concourse.tile as tile

    f32 = mybir.dt.float32
    bf16 = mybir.dt.bfloat16
    fp8 = mybir.dt.float8e4
    Exp = mybir.ActivationFunctionType.Exp
    Square = mybir.ActivationFunctionType.Square
    Copy = mybir.ActivationFunctionType.Copy
    Alu = mybir.AluOpType

    j_full = sum(chunks)
    nchunk = len(chunks)
    offs = [sum(chunks[:k]) for k in range(nchunk)]

    nc = bacc.Bacc("TRN2", target_bir_lowering=False, debug=False,
                   enable_asserts=False)

    lg_d = nc.dram_tensor("logits", [j_full * P * C], bf16,
                          kind="ExternalInput")
    m_d = nc.dram_tensor("m8", [j_full * P * C], fp8, kind="ExternalInput")
    sq_d = nc.dram_tensor("sq_out", [C, F2], f32, kind="ExternalOutput")

    with tile.TileContext(nc) as tc:
        with (
            tc.tile_pool(name="const", bufs=1) as constp,
            tc.tile_pool(name="ld", bufs=4) as ldp,
            tc.tile_pool(name="big", bufs=3) as bigp,
            tc.tile_pool(name="tree", bufs=3) as treep,
            tc.tile_pool(name="small", bufs=4) as smallp,
            tc.tile_pool(name="psum", bufs=1, space="PSUM") as psump,
        ):
            psum_sq = psump.tile([C, F2], f32)

            Ls = [None] * nchunk
            Ms = [None] * nchunk
            Es = [None] * nchunk
            RHs = [None] * nchunk

            def emit_load(k):
                cj = chunks[k]
                base = offs[k] * P * C
                L = ldp.tile([P, cj, C], bf16, tag="L")
                lg_ap = lg_d[base:base + P * C * cj].rearrange(
                    "(p j c) -> p j c", j=cj, c=C)
                nc.sync.dma_start(L[:], lg_ap)
                M8 = ldp.tile([P, cj, C], fp8, tag="M8")
                m_ap = m_d[base:base + P * C * cj].rearrange(
                    "(p j c) -> p j c", j=cj, c=C)
                # scalar-issued DMA rides the second HWDGE ring
                nc.scalar.dma_start(M8[:], m_ap)
                Ls[k], Ms[k] = L, M8

            def emit_exp(k):
                cj = chunks[k]
                E = bigp.tile([P, cj, C], bf16, tag="E")
                nc.scalar.activation(E[:], Ls[k][:], Exp)
                Es[k] = E

            emit_load(0)
            emit_load(1)
            emit_exp(0)
            for k, cj in enumerate(chunks):
                if k + 2 < nchunk:
                    emit_load(k + 2)
                E, M8 = Es[k], Ms[k]

                # d = rowsum(E): bf16 halving tree (2x mode) + packed tail
                h1 = treep.tile([P, cj, 64], bf16, tag="h1")
                nc.vector.tensor_tensor(h1[:], E[:, :, 0:64], E[:, :, 64:128],
                                        op=Alu.add)
                h2 = treep.tile([P, cj, 32], bf16, tag="h2")
                nc.vector.tensor_tensor(h2[:], h1[:, :, 0:32], h1[:, :, 32:64],
                                        op=Alu.add)
                h3 = treep.tile([P, cj, 16], bf16, tag="h3")
                nc.vector.tensor_tensor(h3[:], h2[:, :, 0:16], h2[:, :, 16:32],
                                        op=Alu.add)
                d = smallp.tile([P, cj], f32, tag="d")
                nc.vector.tensor_reduce(d[:], h3[:],
                                        axis=mybir.AxisListType.X, op=Alu.add)
                rv = smallp.tile([P, cj], f32, tag="rv")
                nc.vector.reciprocal_approx_fast(rv[:], d[:])
                rb = smallp.tile([P, cj], bf16, tag="rb")
                nc.scalar.activation(rb[:], rv[:], Copy)

                rhs = bigp.tile([P, cj, 2, C], bf16, tag="rhs")
                nc.vector.tensor_tensor(
                    rhs[:, :, 0, :], E[:],
                    rb[:, :, None].to_broadcast([P, cj, C]),
                    op=Alu.mult)
                # next chunk's exp goes ahead of this chunk's square in the
                # in-order ACT queue so exp is never stalled behind DVE
                if k + 1 < nchunk:
                    emit_exp(k + 1)
                if act_sq[k]:
                    nc.scalar.activation(rhs[:, :, 1, :], rhs[:, :, 0, :],
                                         Square)
                else:
                    nc.vector.tensor_tensor(
                        rhs[:, :, 1, :], rhs[:, :, 0, :], rhs[:, :, 0, :],
                        op=Alu.mult)
                RHs[k] = rhs

                for jj in range(cj):
                    j = offs[k] + jj
                    nc.tensor.matmul(
                        psum_sq[:], M8[:, jj, :], rhs[:, jj, :, :],
                        start=(j == 0), stop=(j == j_full - 1))

            out_t = constp.tile([C, F2], f32)
            nc.vector.tensor_copy(out_t[:], psum_sq[:])
            nc.sync.dma_start(sq_d[:], out_t[:])

    nc.compile()
    return nc


def _get_nc():
    key = (CHUNKS, ACT_SQ)
    if key not in _NC_CACHE:
        _NC_CACHE[key] = build_nc(CHUNKS, ACT_SQ)
    return _NC_CACHE[key]


def _pack_core(Lv, Sv, chunks):
    """Pack one core's [J, P, C] logits + [J, P] segments into the DMA
    layouts: both row-major (p, j, c) per chunk."""
    bf16 = ml_dtypes.bfloat16
    fp8 = ml_dtypes.float8_e4m3
    offs = 0
    lparts = []
    mparts = []
    for cj in chunks:
        Lc = Lv[offs:offs + cj]                      # [cj, P, C]
        lparts.append(np.ascontiguousarray(
            Lc.transpose(1, 0, 2)).ravel())          # (p, j, c)
        M = np.zeros((cj, P, C), dtype=fp8)
        Sc = Sv[offs:offs + cj]                      # [cj, P]
        valid = Sc >= 0
        jj, pp = np.nonzero(valid)
        M[jj, pp, Sc[jj, pp]] = fp8(1.0)
        mparts.append(np.ascontiguousarray(
            M.transpose(1, 0, 2)).ravel())           # (p, j, c)
        offs += cj
    return (np.concatenate(lparts).astype(bf16, copy=False),
            np.concatenate(mparts))


def kernel(column_logits, column_assignments, valid_mask):
    global LAST_RESULT
    from concourse.bass_utils import run_bass_kernel_spmd

    bf16 = ml_dtypes.bfloat16

    logits = np.asarray(column_logits, dtype=np.float32).reshape(N_TOK, C)
    seg = np.asarray(column_assignments).reshape(N_TOK).astype(np.int64)
    w = np.asarray(valid_mask).reshape(N_TOK).astype(bool)

    vidx = np.nonzero(w)[0]
    cap = NCORES * TOK_CAP
    dev_idx = vidx[:cap]
    ov_idx = vidx[cap:]          # overflow (essentially never non-empty)

    nv = dev_idx.size
    # Compacted per-core arrays [J, P, C] / [J, P]; seg = -1 marks padding.
    Lv = np.zeros((cap, C), dtype=bf16)
    Lv[:nv] = logits[dev_idx].astype(bf16)
    Sv = np.full(cap, -1, dtype=np.int64)
    Sv[:nv] = seg[dev_idx]

    in_maps = []
    for i in range(NCORES):
        sl = slice(i * TOK_CAP, (i + 1) * TOK_CAP)
        lpk, mpk = _pack_core(Lv[sl].reshape(J, P, C),
                              Sv[sl].reshape(J, P), CHUNKS)
        in_maps.append({"logits": lpk, "m8": mpk})

    nc = _get_nc()
    res = run_bass_kernel_spmd(nc, in_maps, list(range(NCORES)), trace=TRACE,
                               tmpdir=TRACE_TMPDIR)
    LAST_RESULT = res

    SQ = np.zeros((C, F2), np.float64)
    for rm in res.results:
        SQ += np.asarray(rm["sq_out"], dtype=np.float64)
    S = SQ[:, 0:C].copy()
    Q = SQ[:, C:F2].copy()

    if ov_idx.size:              # exact host fold-in of overflow tokens
        Lo = logits[ov_idx].astype(np.float64)
        Eo = np.exp(Lo)
        po = Eo / Eo.sum(axis=1, keepdims=True)
        so = seg[ov_idx]
        np.add.at(S, so, po)
        np.add.at(Q, so, po * po)

    n = np.bincount(seg[w], minlength=C).astype(np.float64)
    n_safe = np.maximum(n, 1.0)
    ssd_sum = Q.sum(axis=1) - (S * S).sum(axis=1) / n_safe
    col_var = ssd_sum / (n_safe * C)
    has_multi = n > 1.0
    count = has_multi.sum()
    total = np.where(has_multi, col_var, 0.0).sum()
    loss = total / max(count, 1.0) if count > 0 else 0.0
    return np.asarray(loss, dtype=np.float32)


# revision 4
# speedup vs baseline: 1.3064x; 1.0340x over previous
"""Trainium2 Bass kernel for ColumnConsistencyLoss (segment_reduce).

Problem: B=16, T=8192, C=128.
  probs = softmax(logits, -1)           # (N, C), N = B*T = 131072
  per column-id c (segment): n_c = #valid tokens, S_c = sum w*p, Q_c = sum w*p^2
  col_var_c = (sum_j Q_cj - sum_j S_cj^2 / n_safe_c) / (n_safe_c * C)
  loss = mean over columns with n_c > 1 of col_var_c

Sharding ("compacted"): only VALID tokens (w=1, ~50% of N) contribute to
the loss, so the host gathers the valid tokens and distributes them
evenly over the 8 cores (any token->core mapping is legal: the segment
sums are permutation invariant).  Each core gets a fixed capacity of
J=65 tiles x 128 tokens = 8320 tokens (valid count is ~8192 +- 23 per
core); unused slots are padded with zero logits and a zero one-hot
column so they contribute nothing.  In the (astronomically unlikely)
case that the valid count exceeds the total capacity, the overflow
tokens are folded in exactly on the host.

Device kernel per core (tokens on partitions, row-major [P, j, C] tiles
so the matmul moving operand is contiguous; whole-core SBUF buffers so
there is no pool-rotation serialization):
  - DMA L [P, J, C] bf16 in 4 pieces (sync ring), M [P, J, C] fp8
    one-hot in 1 piece (gpsimd ring, off the compute engines)
  - ACT:  E = exp(L) per chunk        (one big-free instruction each)
  - DVE:  d = rowsum(E) via bf16 halving tree h1,h2,h3 (2x mode) +
          f32 reduce over the remaining 16
  - DVE:  rv = 1/d (reciprocal_approx_fast); ACT: rb2 = bf16(rv) x2
          (the normalizer is stored as an adjacent PAIR so the
          broadcast multiply below can use the DVE 2x mode: all
          operands are 2-byte with a packed stride-1 last dim)
  - DVE:  rhs[.., 0, :] = E * pairbcast(rb2)    (normalized probs p)
  - ACT or DVE (per-chunk knob): rhs[.., 1, :] = p^2
  - PE:   per tile jj: psum[c, 0:2C] += M[:,jj,:]^T @ rhs[:, jj, :, :]
          accumulating S rows (F 0:C) and Q rows (F C:2C) over all tiles
Host finalizes: exact n via bincount, ssd = rowsum(Q) - rowsum(S^2)/n,
masked mean over columns with n>1.
"""

import numpy as np
import ml_dtypes

NCORES = 8
P = 128           # partitions (tokens per tile)
C = 128           # columns / segments
H = C // 2
B, T = 16, 8192
N_TOK = B * T
CHUNKS = (4, 8, 12, 14, 14, 13)       # token tiles per compute chunk
ACT_SQ = (False, True, True, True, False, False)  # square on ACT?
DMA_PIECES = (6, 16, 22, 21)          # token tiles per L-DMA piece
J = sum(CHUNKS)                        # 65 tiles
TOK_CAP = J * P                        # 8320 tokens per core
F2 = 2 * C

TRACE = False          # set True (e.g. from test.py) to capture NTFF profile
TRACE_TMPDIR = None    # where trace/NEFF artifacts land when TRACE is set
LAST_RESULT = None     # BassKernelResults of the last run (for profiling)

_NC_CACHE = {}


def build_nc(chunks=CHUNKS, act_sq=ACT_SQ, dma_pieces=DMA_PIECES):
    """Build + compile the Bass program (SPMD; same NEFF on all cores)."""
    from concourse import bacc, mybir
    import concourse.tile as tile

    f32 = mybir.dt.float32
    bf16 = mybir.dt.bfloat16
    fp8 = mybir.dt.float8e4
    Exp = mybir.ActivationFunctionType.Exp
    Square = mybir.ActivationFunctionType.Square
    Copy = mybir.ActivationFunctionType.Copy
    Alu = mybir.AluOpType

    j_full = sum(chunks)
    assert sum(dma_pieces) == j_full
    nchunk = len(chunks)
    offs = [sum(chunks[:k]) for k in range(nchunk)]

    nc = bacc.Bacc("TRN2", target_bir_lowering=False, debug=False,
                   enable_asserts=False)

    lg_d = nc.dram_tensor("logits", [P * j_full * C], bf16,
                          kind="ExternalInput")
    m_d = nc.dram_tensor("m8", [P * j_full * C], fp8, kind="ExternalInput")
    sq_d = nc.dram_tensor("sq_out", [C, F2], f32, kind="ExternalOutput")

    lg_ap = lg_d[:].rearrange("(p j c) -> p j c", j=j_full, c=C)
    m_ap = m_d[:].rearrange("(p j c) -> p j c", j=j_full, c=C)

    with tile.TileContext(nc) as tc:
        with (
            tc.tile_pool(name="buf", bufs=1) as bufp,
            tc.tile_pool(name="psum", bufs=1, space="PSUM") as psump,
        ):
            psum_sq = psump.tile([C, F2], f32)

            L = bufp.tile([P, j_full, C], bf16)
            M8 = bufp.tile([P, j_full, C], fp8)
            E = bufp.tile([P, j_full, C], bf16)
            RHS = bufp.tile([P, j_full, 2, C], bf16)
            h1 = bufp.tile([P, j_full, H], bf16)
            h2 = bufp.tile([P, j_full, 32], bf16)
            h3 = bufp.tile([P, j_full, 16], bf16)
            dd = bufp.tile([P, j_full], f32)
            rv = bufp.tile([P, j_full], f32)
            rb2 = bufp.tile([P, j_full, 2], bf16)

            # the one-hot goes on the gpsimd (SWDGE) ring: it is not
            # needed until the first matmul and keeps the compute
            # engines' queues free of DGE dispatch work
            nc.gpsimd.dma_start(M8[:], m_ap)
            doffs = [sum(dma_pieces[:k]) for k in range(len(dma_pieces))]
            for a, cj in zip(doffs, dma_pieces):
                nc.sync.dma_start(L[:, a:a + cj, :], lg_ap[:, a:a + cj, :])

            def pair(ap):  # [P, cj, C] -> [P, cj, 64, 2] (packed pairs)
                return ap.rearrange("p j (h t) -> p j h t", t=2)

            nc.scalar.activation(E[:, 0:chunks[0], :], L[:, 0:chunks[0], :],
                                 Exp)
            for k, cj in enumerate(chunks):
                a, b = offs[k], offs[k] + cj
                # d = rowsum(E): bf16 halving tree (2x mode) + packed tail
                nc.vector.tensor_tensor(h1[:, a:b, :], E[:, a:b, 0:H],
                                        E[:, a:b, H:C], op=Alu.add)
                nc.vector.tensor_tensor(h2[:, a:b, :], h1[:, a:b, 0:32],
                                        h1[:, a:b, 32:64], op=Alu.add)
                nc.vector.tensor_tensor(h3[:, a:b, :], h2[:, a:b, 0:16],
                                        h2[:, a:b, 16:32], op=Alu.add)
                nc.vector.tensor_reduce(dd[:, a:b], h3[:, a:b, :],
                                        axis=mybir.AxisListType.X, op=Alu.add)
                nc.vector.reciprocal_approx_fast(rv[:, a:b], dd[:, a:b])
                nc.scalar.activation(
                    rb2[:, a:b, :],
                    rv[:, a:b, None].to_broadcast([P, cj, 2]), Copy)

                nc.vector.tensor_tensor(
                    pair(RHS[:, a:b, 0, :]), pair(E[:, a:b, :]),
                    rb2[:, a:b, None, :].to_broadcast([P, cj, H, 2]),
                    op=Alu.mult)
                # next chunk's exp goes ahead of this chunk's square in the
                # in-order ACT queue so exp is never stalled behind DVE
                if k + 1 < nchunk:
                    a2, b2 = offs[k + 1], offs[k + 1] + chunks[k + 1]
                    nc.scalar.activation(E[:, a2:b2, :], L[:, a2:b2, :], Exp)
                if act_sq[k]:
                    nc.scalar.activation(RHS[:, a:b, 1, :], RHS[:, a:b, 0, :],
                                         Square)
                else:
                    nc.vector.tensor_tensor(
                        RHS[:, a:b, 1, :], RHS[:, a:b, 0, :],
                        RHS[:, a:b, 0, :], op=Alu.mult)

                for jj in range(a, b):
                    nc.tensor.matmul(
                        psum_sq[:], M8[:, jj, :], RHS[:, jj, :, :],
                        start=(jj == 0), stop=(jj == j_full - 1))

            out_t = bufp.tile([C, F2], f32)
            nc.vector.tensor_copy(out_t[:], psum_sq[:])
            nc.sync.dma_start(sq_d[:], out_t[:])

    nc.compile()
    return nc


def _get_nc():
    key = (CHUNKS, ACT_SQ, DMA_PIECES)
    if key not in _NC_CACHE:
        _NC_CACHE[key] = build_nc(CHUNKS, ACT_SQ, DMA_PIECES)
    return _NC_CACHE[key]


def _pack_core(Lv, Sv):
    """Pack one core's [J, P, C] logits + [J, P] segments into the
    (p, j, c) DMA layout."""
    bf16 = ml_dtypes.bfloat16
    fp8 = ml_dtypes.float8_e4m3
    lpk = np.ascontiguousarray(Lv.transpose(1, 0, 2)).ravel()
    M = np.zeros((J, P, C), dtype=fp8)
    valid = Sv >= 0
    jj, pp = np.nonzero(valid)
    M[jj, pp, Sv[jj, pp]] = fp8(1.0)
    mpk = np.ascontiguousarray(M.transpose(1, 0, 2)).ravel()
    return lpk.astype(bf16, copy=False), mpk


def kernel(column_logits, column_assignments, valid_mask):
    global LAST_RESULT
    from concourse.bass_utils import run_bass_kernel_spmd

    bf16 = ml_dtypes.bfloat16

    logits = np.asarray(column_logits, dtype=np.float32).reshape(N_TOK, C)
    seg = np.asarray(column_assignments).reshape(N_TOK).astype(np.int64)
    w = np.asarray(valid_mask).reshape(N_TOK).astype(bool)

    vidx = np.nonzero(w)[0]
    cap = NCORES * TOK_CAP
    dev_idx = vidx[:cap]
    ov_idx = vidx[cap:]          # overflow (essentially never non-empty)

    nv = dev_idx.size
    # Compacted per-core arrays [J, P, C] / [J, P]; seg = -1 marks padding.
    Lv = np.zeros((cap, C), dtype=bf16)
    Lv[:nv] = logits[dev_idx].astype(bf16)
    Sv = np.full(cap, -1, dtype=np.int64)
    Sv[:nv] = seg[dev_idx]

    in_maps = []
    for i in range(NCORES):
        sl = slice(i * TOK_CAP, (i + 1) * TOK_CAP)
        lpk, mpk = _pack_core(Lv[sl].reshape(J, P, C),
                              Sv[sl].reshape(J, P))
        in_maps.append({"logits": lpk, "m8": mpk})

    nc = _get_nc()
    res = run_bass_kernel_spmd(nc, in_maps, list(range(NCORES)), trace=TRACE,
                               tmpdir=TRACE_TMPDIR)
    LAST_RESULT = res

    SQ = np.zeros((C, F2), np.float64)
    for rm in res.results:
        SQ += np.asarray(rm["sq_out"], dtype=np.float64)
    S = SQ[:, 0:C].copy()
    Q = SQ[:, C:F2].copy()

    if ov_idx.size:              # exact host fold-in of overflow tokens
        Lo = logits[ov_idx].astype(np.float64)
        Eo = np.exp(Lo)
        po = Eo / Eo.sum(axis=1, keepdims=True)
        so = seg[ov_idx]
        np.add.at(S, so, po)
        np.add.at(Q, so, po * po)

    n = np.bincount(seg[w], minlength=C).astype(np.float64)
    n_safe = np.maximum(n, 1.0)
    ssd_sum = Q.sum(axis=1) - (S * S).sum(axis=1) / n_safe
    col_var = ssd_sum / (n_safe * C)
    has_multi = n > 1.0
    count = has_multi.sum()
    total = np.where(has_multi, col_var, 0.0).sum()
    loss = total / max(count, 1.0) if count > 0 else 0.0
    return np.asarray(loss, dtype=np.float32)


# revision 9
# speedup vs baseline: 1.4461x; 1.1070x over previous
"""Trainium2 Bass kernel for ColumnConsistencyLoss (segment_reduce).

Problem: B=16, T=8192, C=128.
  probs = softmax(logits, -1)           # (N, C), N = B*T = 131072
  per column-id c (segment): n_c = #valid tokens, S_c = sum w*p, Q_c = sum w*p^2
  col_var_c = (sum_j Q_cj - sum_j S_cj^2 / n_safe_c) / (n_safe_c * C)
  loss = mean over columns with n_c > 1 of col_var_c

Sharding ("compacted"): only VALID tokens (w=1, ~50% of N) contribute to
the loss, so the host gathers the valid tokens and distributes them
evenly over the 8 cores (any token->core mapping is legal: the segment
sums are permutation invariant).  Each core gets a fixed capacity of
J=65 tiles x 128 tokens = 8320 tokens (valid count is ~8192 +- 23 per
core); unused slots are padded with zero logits and a zero one-hot
column so they contribute nothing.  In the (astronomically unlikely)
case that the valid count exceeds the total capacity, the overflow
tokens are folded in exactly on the host.

Device kernel per core (tokens on partitions, row-major [P, j, C] tiles
so the matmul moving operand is contiguous; whole-core SBUF buffers so
there is no pool-rotation serialization):
  - DMA L [P, J, C] bf16 in 4 pieces (sync ring), M [P, J, C] fp8
    one-hot in 1 piece (gpsimd ring, off the compute engines)
  - ACT:  E = exp(L) per chunk        (one big-free instruction each)
  - DVE:  d = rowsum(E) via bf16 halving tree h1,h2,h3 (2x mode) +
          f32 reduce over the remaining 16
  - DVE:  rv = 1/d (reciprocal_approx_fast); ACT: rb2 = bf16(rv) x2
          (the normalizer is stored as an adjacent PAIR so the
          broadcast multiply below can use the DVE 2x mode: all
          operands are 2-byte with a packed stride-1 last dim)
  - DVE:  rhs[.., 0, :] = E * pairbcast(rb2)    (normalized probs p)
  - ACT or DVE (per-chunk knob): rhs[.., 1, :] = p^2
  - PE:   per tile jj: psum[c, 0:2C] += M[:,jj,:]^T @ rhs[:, jj, :, :]
          accumulating S rows (F 0:C) and Q rows (F C:2C) over all tiles
Host finalizes: exact n via bincount, ssd = rowsum(Q) - rowsum(S^2)/n,
masked mean over columns with n>1.
"""

import numpy as np
import ml_dtypes

NCORES = 8
P = 128           # partitions (tokens per tile)
C = 128           # columns / segments
H = C // 2
B, T = 16, 8192
N_TOK = B * T
CHUNKS = (2, 6, 12, 15, 15, 15)       # token tiles per compute chunk
ACT_SQ = (False, False, True, True, True, False)  # square on ACT?
M_SPLIT = 24                           # one-hot DMA piece boundary (tiles)
L_FP8 = True                           # stream logits as fp8e4m3 (rel err ~6e-3)
J = sum(CHUNKS)                        # 65 tiles
TOK_CAP = J * P                        # 8320 tokens per core
F2 = 2 * C

TRACE = False          # set True (e.g. from test.py) to capture NTFF profile
TRACE_TMPDIR = None    # where trace/NEFF artifacts land when TRACE is set
LAST_RESULT = None     # BassKernelResults of the last run (for profiling)

_NC_CACHE = {}


def build_nc(chunks=CHUNKS, act_sq=ACT_SQ, m_split=M_SPLIT, l_fp8=L_FP8):
    """Build + compile the Bass program (SPMD; same NEFF on all cores)."""
    from concourse import bacc, mybir
    import concourse.tile as tile

    f32 = mybir.dt.float32
    bf16 = mybir.dt.bfloat16
    fp8 = mybir.dt.float8e4
    ldt = fp8 if l_fp8 else bf16
    Exp = mybir.ActivationFunctionType.Exp
    Square = mybir.ActivationFunctionType.Square
    Alu = mybir.AluOpType

    j_full = sum(chunks)
    nchunk = len(chunks)
    offs = [sum(chunks[:k]) for k in range(nchunk)]

    nc = bacc.Bacc("TRN2", target_bir_lowering=False, debug=False,
                   enable_asserts=False)

    lg_d = nc.dram_tensor("logits", [P * j_full * C], ldt,
                          kind="ExternalInput")
    m_d = nc.dram_tensor("m8", [P * j_full * C], fp8, kind="ExternalInput")
    sq_d = nc.dram_tensor("sq_out", [C, F2], f32, kind="ExternalOutput")

    lg_ap = lg_d[:].rearrange("(p j c) -> p j c", j=j_full, c=C)
    m_ap = m_d[:].rearrange("(p j c) -> p j c", j=j_full, c=C)

    with tile.TileContext(nc) as tc:
        with (
            tc.tile_pool(name="buf", bufs=1) as bufp,
            tc.tile_pool(name="psum", bufs=1, space="PSUM") as psump,
        ):
            psum_sq = psump.tile([C, F2], f32)

            L = bufp.tile([P, j_full, C], ldt)
            M8 = bufp.tile([P, j_full, C], fp8)
            E = bufp.tile([P, j_full, C], bf16)
            RHS = bufp.tile([P, j_full, 2, C], bf16)
            h1 = bufp.tile([P, j_full, H], bf16)
            h2 = bufp.tile([P, j_full, 32], bf16)
            h3 = bufp.tile([P, j_full, 16], bf16)
            dd = bufp.tile([P, j_full], f32)
            rv = bufp.tile([P, j_full], f32)
            rb2 = bufp.tile([P, j_full, 2], bf16)
            junk = bufp.tile([P, 2], f32)

            # Warm-ups on garbage data while the DMA is in flight: pull
            # the ACT exp-table load and the DVE custom-op library load
            # off the critical path.
            nc.scalar.activation(junk[:], junk[:], Exp)
            nc.vector.reciprocal_approx_fast(junk[:], junk[:])

            # All input DMA rides ONE HWDGE ring (sync) so arrival order
            # is exactly program order: logits pieces (chunk-aligned,
            # small first so compute starts early) with the one-hot
            # pieces interleaved just-in-time for the matmuls.
            for k in range(nchunk):
                a, b = offs[k], offs[k] + chunks[k]
                nc.sync.dma_start(L[:, a:b, :], lg_ap[:, a:b, :])
                if k == 1:
                    nc.sync.dma_start(M8[:, 0:m_split, :],
                                      m_ap[:, 0:m_split, :])
                if k == 3:
                    nc.sync.dma_start(M8[:, m_split:j_full, :],
                                      m_ap[:, m_split:j_full, :])

            def pair(ap):  # [P, cj, C] -> [P, cj, 64, 2] (packed pairs)
                return ap.rearrange("p j (h t) -> p j h t", t=2)

            nc.scalar.activation(E[:, 0:chunks[0], :], L[:, 0:chunks[0], :],
                                 Exp)
            for k, cj in enumerate(chunks):
                a, b = offs[k], offs[k] + cj
                # d = rowsum(E): bf16 halving tree (2x mode) + packed tail
                nc.vector.tensor_tensor(h1[:, a:b, :], E[:, a:b, 0:H],
                                        E[:, a:b, H:C], op=Alu.add)
                nc.vector.tensor_tensor(h2[:, a:b, :], h1[:, a:b, 0:32],
                                        h1[:, a:b, 32:64], op=Alu.add)
                nc.vector.tensor_tensor(h3[:, a:b, :], h2[:, a:b, 0:16],
                                        h2[:, a:b, 16:32], op=Alu.add)
                nc.vector.tensor_reduce(dd[:, a:b], h3[:, a:b, :],
                                        axis=mybir.AxisListType.X, op=Alu.add)
                nc.vector.reciprocal_approx_fast(rv[:, a:b], dd[:, a:b])
                nc.gpsimd.tensor_copy(
                    rb2[:, a:b, :],
                    rv[:, a:b, None].to_broadcast([P, cj, 2]))

                nc.vector.tensor_tensor(
                    pair(RHS[:, a:b, 0, :]), pair(E[:, a:b, :]),
                    rb2[:, a:b, None, :].to_broadcast([P, cj, H, 2]),
                    op=Alu.mult)
                # next chunk's exp goes ahead of this chunk's square in the
                # in-order ACT queue so exp is never stalled behind DVE
                if k + 1 < nchunk:
                    a2, b2 = offs[k + 1], offs[k + 1] + chunks[k + 1]
                    nc.scalar.activation(E[:, a2:b2, :], L[:, a2:b2, :], Exp)
                if act_sq[k]:
                    nc.scalar.activation(RHS[:, a:b, 1, :], RHS[:, a:b, 0, :],
                                         Square)
                else:
                    nc.vector.tensor_tensor(
                        RHS[:, a:b, 1, :], RHS[:, a:b, 0, :],
                        RHS[:, a:b, 0, :], op=Alu.mult)

                for jj in range(a, b):
                    nc.tensor.matmul(
                        psum_sq[:], M8[:, jj, :], RHS[:, jj, :, :],
                        start=(jj == 0), stop=(jj == j_full - 1))

            out_t = bufp.tile([C, F2], f32)
            nc.vector.tensor_copy(out_t[:], psum_sq[:])
            nc.sync.dma_start(sq_d[:], out_t[:])

    nc.compile()
    return nc


def _get_nc():
    key = (CHUNKS, ACT_SQ, M_SPLIT, L_FP8)
    if key not in _NC_CACHE:
        _NC_CACHE[key] = build_nc(CHUNKS, ACT_SQ, M_SPLIT, L_FP8)
    return _NC_CACHE[key]


def _pack_core(Lv, Sv):
    """Pack one core's [J, P, C] logits + [J, P] segments into the
    (p, j, c) DMA layout."""
    fp8 = ml_dtypes.float8_e4m3
    lpk = np.ascontiguousarray(Lv.transpose(1, 0, 2)).ravel()
    M = np.zeros((J, P, C), dtype=fp8)
    valid = Sv >= 0
    jj, pp = np.nonzero(valid)
    M[jj, pp, Sv[jj, pp]] = fp8(1.0)
    mpk = np.ascontiguousarray(M.transpose(1, 0, 2)).ravel()
    return lpk, mpk


def kernel(column_logits, column_assignments, valid_mask):
    global LAST_RESULT
    from concourse.bass_utils import run_bass_kernel_spmd

    ldt = ml_dtypes.float8_e4m3 if L_FP8 else ml_dtypes.bfloat16

    logits = np.asarray(column_logits, dtype=np.float32).reshape(N_TOK, C)
    seg = np.asarray(column_assignments).reshape(N_TOK).astype(np.int64)
    w = np.asarray(valid_mask).reshape(N_TOK).astype(bool)

    vidx = np.nonzero(w)[0]
    cap = NCORES * TOK_CAP
    dev_idx = vidx[:cap]
    ov_idx = vidx[cap:]          # overflow (essentially never non-empty)

    nv = dev_idx.size
    # Compacted per-core arrays [J, P, C] / [J, P]; seg = -1 marks padding.
    Lv = np.zeros((cap, C), dtype=ldt)
    Lv[:nv] = logits[dev_idx].astype(ldt)
    Sv = np.full(cap, -1, dtype=np.int64)
    Sv[:nv] = seg[dev_idx]

    in_maps = []
    for i in range(NCORES):
        sl = slice(i * TOK_CAP, (i + 1) * TOK_CAP)
        lpk, mpk = _pack_core(Lv[sl].reshape(J, P, C),
                              Sv[sl].reshape(J, P))
        in_maps.append({"logits": lpk, "m8": mpk})

    nc = _get_nc()
    res = run_bass_kernel_spmd(nc, in_maps, list(range(NCORES)), trace=TRACE,
                               tmpdir=TRACE_TMPDIR)
    LAST_RESULT = res

    SQ = np.zeros((C, F2), np.float64)
    for rm in res.results:
        SQ += np.asarray(rm["sq_out"], dtype=np.float64)
    S = SQ[:, 0:C].copy()
    Q = SQ[:, C:F2].copy()

    if ov_idx.size:              # exact host fold-in of overflow tokens
        Lo = logits[ov_idx].astype(np.float64)
        Eo = np.exp(Lo)
        po = Eo / Eo.sum(axis=1, keepdims=True)
        so = seg[ov_idx]
        np.add.at(S, so, po)
        np.add.at(Q, so, po * po)

    n = np.bincount(seg[w], minlength=C).astype(np.float64)
    n_safe = np.maximum(n, 1.0)
    ssd_sum = Q.sum(axis=1) - (S * S).sum(axis=1) / n_safe
    col_var = ssd_sum / (n_safe * C)
    has_multi = n > 1.0
    count = has_multi.sum()
    total = np.where(has_multi, col_var, 0.0).sum()
    loss = total / max(count, 1.0) if count > 0 else 0.0
    return np.asarray(loss, dtype=np.float32)
